# revision 12
# baseline (speedup 1.0000x reference)
"""Trainium2 Bass kernel for sparse submanifold conv net (nn_Net2_38963943309314).

Pipeline: SubMConv3d(3->256) -> SubMConv3d(256->256) -> SparseMaxPool3d(2)
          -> SubMConv3d(256->512) -> SubMConv3d(512->512)
over 50k active voxels in a 40x1024x1024 grid (density ~0.1%).

Strategy: voxel density is so low that the 27-offset rulebook is dominated by
the center (self) offset.  Each conv = dense center GEMM over rows + a small
number of sparse neighbor "correction" pairs.  Data-parallel over 8 cores:
each core owns a contiguous slab of final (pooled) rows and redundantly
computes the halo cone it needs (no collectives).  Corrections are computed
k-grouped (gather rows -> PE transpose -> GEMM with w[k]) scattered into
per-output-tile slots (V'), then folded into the main GEMM PSUM accumulation
with host-built one-hot S matmuls.  Pool = row-aliasing + an indirect
gather/max/scatter fixup for the ~177 two-child cells.  All matmuls run in
float32r (TF32-like) at full PE rate.
"""

import math
import sys

import numpy as np
from itertools import product

D, H, W = 40, 1024, 1024
OFF3 = tuple(product((-1, 0, 1), repeat=3))
OFF2 = tuple(product((0, 1), repeat=3))
KC = 13  # center offset index in OFF3
NCORES = 8
P = 128


# ---------------------------------------------------------------- host: rulebook
def _build_grid(coors, shape):
    d, h, w = shape
    size = d * h * w
    lin = (coors[:, 0].astype(np.int64) * h + coors[:, 1]) * w + coors[:, 2]
    g = np.full(size + 1, -1, np.int32)
    g[lin] = np.arange(coors.shape[0], dtype=np.int32)
    return g


def _conv_rules(coors, grid, shape):
    d, h, w = shape
    size = d * h * w
    rules = []
    for dz, dy, dx in OFF3:
        nz, ny, nx = coors[:, 0] + dz, coors[:, 1] + dy, coors[:, 2] + dx
        ok = (nz >= 0) & (nz < d) & (ny >= 0) & (ny < h) & (nx >= 0) & (nx < w)
        lin = np.where(ok, (nz.astype(np.int64) * h + ny) * w + nx, size)
        rules.append(grid[lin])
    return np.stack(rules)


def _round_up(x, m):
    return ((x + m - 1) // m) * m


def build_structures(coors):
    """Global rulebooks + pooling structure (mirrors reference.py semantics)."""
    N = coors.shape[0]
    grid1 = _build_grid(coors, (D, H, W))
    r1 = _conv_rules(coors, grid1, (D, H, W))

    d2, h2, w2 = D // 2, H // 2, W // 2
    pkey = ((coors[:, 0] // 2).astype(np.int64) * h2 + coors[:, 1] // 2) * w2 + (
        coors[:, 2] // 2
    )
    ukeys = np.unique(pkey)  # sorted, matches jnp.unique order
    Np = len(ukeys)
    pz = (ukeys // (h2 * w2)).astype(np.int32)
    py = ((ukeys // w2) % h2).astype(np.int32)
    px = (ukeys % w2).astype(np.int32)
    pcoors = np.stack([pz, py, px], 1)
    grid2 = _build_grid(pcoors, (d2, h2, w2))
    prules = []
    for dz, dy, dx in OFF2:
        cz, cy, cx = pz * 2 + dz, py * 2 + dy, px * 2 + dx
        lin = (cz.astype(np.int64) * H + cy) * W + cx
        prules.append(grid1[lin])
    prules = np.stack(prules)  # [8, Np] voxel-row ids, -1 inactive
    r2 = _conv_rules(pcoors, grid2, (d2, h2, w2))
    return dict(N=N, r1=r1, r2=r2, Np=Np, pcoors=pcoors, prules=prules)


def _pairs_for(rows, rules):
    """(i_local, j_global, k) for all non-center active pairs with i in rows."""
    out_i, out_j, out_k = [], [], []
    for k in range(27):
        if k == KC:
            continue
        j = rules[k][rows]
        m = j >= 0
        ii = np.nonzero(m)[0]
        out_i.append(ii.astype(np.int32))
        out_j.append(j[m].astype(np.int32))
        out_k.append(np.full(ii.shape, k, np.int32))
    return np.concatenate(out_i), np.concatenate(out_j), np.concatenate(out_k)


def _extend(rows, extra_global):
    """rows(list) + sorted new rows from extra_global not already present."""
    s = set(rows.tolist())
    new = sorted(set(extra_global.tolist()) - s)
    return np.concatenate([rows, np.asarray(new, np.int32)]).astype(np.int32)


class CoreStruct:
    pass


def build_core(st, c, slab):
    """Per-core row spaces + pair lists (global indices still)."""
    cs = CoreStruct()
    Np = st["Np"]
    r1, r2, prules = st["r1"], st["r2"], st["prules"]
    own = np.arange(c * slab, min((c + 1) * slab, Np), dtype=np.int32)
    cs.n_own = len(own)

    i4, j4, k4 = _pairs_for(own, r2)
    L3 = _extend(own, j4)  # x3 rows (pooled ids)
    cs.L3 = L3
    i3, j3, k3 = _pairs_for(L3, r2)
    L2p = _extend(L3, j3)  # x2p rows (pooled ids)
    cs.L2p = L2p

    inv3 = {g: i for i, g in enumerate(L3.tolist())}
    inv2p = {g: i for i, g in enumerate(L2p.tolist())}
    cs.pairs4 = (i4, np.array([inv3[g] for g in j4.tolist()], np.int32), k4)
    cs.pairs3 = (i3, np.array([inv2p[g] for g in j3.tolist()], np.int32), k3)

    # pool children for each x2p row
    ch = prules[:, L2p]  # [8, n2p] voxel ids
    valid = ch >= 0
    first_k = np.argmax(valid, 0)
    child0 = ch[first_k, np.arange(ch.shape[1])]  # voxel id per x2p row
    assert (valid.any(0)).all()
    ex_r, ex_v = [], []
    for kk in range(8):
        m = valid[kk] & (kk != first_k)
        rr = np.nonzero(m)[0]
        ex_r.extend(rr.tolist())
        ex_v.extend(ch[kk][rr].tolist())
    cs.fix_targets = np.asarray(ex_r, np.int32)  # x2-local row (region idx)
    cs.extra_vox = np.asarray(ex_v, np.int32)  # voxel ids of 2nd+ children
    cs.child0 = child0.astype(np.int32)
    return cs


def finish_core(st, cs, sizes):
    """Voxel-space row layouts (need uniform M3 first) + conv1/2 pairs."""
    r1 = st["r1"]
    M3, M2, M1, M0 = sizes["M3"], sizes["M2"], sizes["M1"], sizes["M0"]
    n2p = len(cs.L2p)
    nex = len(cs.extra_vox)
    # x2 voxel layout: [0:n2p) = child0; [M3 : M3+nex) = extras; pads dummy
    x2vox = np.zeros(M2, np.int32)
    x2vox[:n2p] = cs.child0
    x2vox[n2p:M3] = cs.child0[0]
    x2vox[M3 : M3 + nex] = cs.extra_vox
    x2vox[M3 + nex :] = cs.child0[0]
    x2real = np.zeros(M2, bool)
    x2real[:n2p] = True
    x2real[M3 : M3 + nex] = True

    inv_x2 = {}
    for r in np.nonzero(x2real)[0].tolist():
        inv_x2[int(x2vox[r])] = r

    # conv2 pairs over real x2 rows
    rr = np.nonzero(x2real)[0]
    i2, j2v, k2 = _pairs_for(x2vox[rr], r1)
    i2 = rr[i2].astype(np.int32)
    # x1 layout = x2 layout [0:M2) + halo region [M2:M1)
    halo2 = sorted(set(j2v.tolist()) - set(inv_x2.keys()))
    assert len(halo2) <= M1 - M2, (len(halo2), M1, M2)
    inv_x1 = dict(inv_x2)
    x1vox = np.concatenate([x2vox, np.zeros(M1 - M2, np.int32)])
    x1real = np.concatenate([x2real, np.zeros(M1 - M2, bool)])
    for t, v in enumerate(halo2):
        inv_x1[int(v)] = M2 + t
        x1vox[M2 + t] = v
        x1real[M2 + t] = True
    x1vox[M2 + len(halo2) :] = x1vox[0]
    j2 = np.array([inv_x1[int(v)] for v in j2v], np.int32)

    # conv1 pairs over real x1 rows
    rr1 = np.nonzero(x1real)[0]
    i1, j1v, k1 = _pairs_for(x1vox[rr1], r1)
    i1 = rr1[i1].astype(np.int32)
    halo1 = sorted(set(j1v.tolist()) - set(inv_x1.keys()))
    assert len(halo1) <= M0 - M1
    inv_f = dict(inv_x1)
    fvox = np.concatenate([x1vox, np.zeros(M0 - M1, np.int32)])
    for t, v in enumerate(halo1):
        inv_f[int(v)] = M1 + t
        fvox[M1 + t] = v
    fvox[M1 + len(halo1) :] = fvox[0]
    j1 = np.array([inv_f[int(v)] for v in j1v], np.int32)

    cs.x2vox, cs.x1vox, cs.fvox = x2vox, x1vox, fvox
    cs.pairs2 = (i2, j2, k2)
    cs.pairs1 = (i1, j1, k1)
    # fixup rows: gather extra rows at [M3+e], target rows cs.fix_targets
    cs.fix_extra_rows = (M3 + np.arange(nex)).astype(np.int32)


# ------------------------------------------------------- host: layer layout data
class LayerPlan:
    """Uniform (across cores) correction layout for one conv layer."""

    def __init__(self, name, Cin, Cout, Mout, pairs_by_core, extra_pad_groups=0):
        self.name, self.Cin, self.Cout, self.Mout = name, Cin, Cout, Mout
        self.ntiles = Mout // P
        # slot size per k-group
        maxk = 1
        maxtile = 1
        for (pi, pj, pk) in pairs_by_core:
            if len(pk):
                maxk = max(maxk, int(np.bincount(pk, minlength=27).max()))
                maxtile = max(maxtile, int(np.bincount(pi // P, minlength=self.ntiles).max()))
        slot = 32
        while slot < maxk:
            slot *= 2
        assert slot <= 256, f"{name}: k-group too big ({maxk})"
        self.SLOT = slot
        pf = 32
        while pf < maxtile:
            pf *= 2
        assert pf <= 512, f"{name}: too many pairs in one out-tile ({maxtile})"
        self.PF = pf
        self.NS = max(1, pf // P)  # S-matmul slices per out-tile (K<=128 each)
        if slot < P:
            gpc = P // slot
            self.NG = _round_up(26, gpc)
            self.chunks = self.NG // gpc
            self.gpc = gpc
        else:
            self.NG = 26
            self.gpc = 1
            self.chunks = 26 * (slot // P)
        self.ks = [k for k in range(27) if k != KC] + [0] * (self.NG - 26)
        self.vrows = self.ntiles * self.PF  # + trash row region below

    def per_core(self, pairs, w_l):
        """gidx/vi/S arrays for one core. pairs=(i_loc,j_loc,k).

        gidx: per V-chunk-slot source row to gather (k-grouped).
        vi:   per (out-tile, PF-slot) -> V-chunk-slot to gather (dummy -> zero
              row at chunks*P).
        S:    one-hot scatter matrices per out-tile.
        """
        pi, pj, pk = pairs
        nslots = self.chunks * P
        gidx = np.zeros((nslots, 1), np.int32)
        vi = np.full((self.ntiles * self.PF, 1), nslots, np.int32)
        S = np.zeros((self.ntiles * self.PF, P), np.float32)
        fill_tile = np.zeros(self.ntiles, np.int32)
        order = np.argsort(pk, kind="stable")
        used = np.zeros(self.NG, np.int32)
        kpos = {k: g for g, k in enumerate(self.ks[:26])}
        for t in order:
            k = int(pk[t])
            g = kpos[k]
            s = g * self.SLOT + used[g]
            used[g] += 1
            gidx[s, 0] = pj[t]
            tile = int(pi[t]) // P
            fs = fill_tile[tile]
            assert fs < self.PF
            fill_tile[tile] += 1
            vslot = tile * self.PF + fs
            vi[vslot, 0] = s
            S[vslot, int(pi[t]) % P] = 1.0
        wv = np.zeros((self.NG * self.Cin, self.Cout), np.float32)
        for g in range(26):
            wv[g * self.Cin : (g + 1) * self.Cin] = w_l[self.ks[g]]
        return gidx, vi, S, wv


# ---------------------------------------------------------------- bass program
def build_program(sz, reps=1):
    sys.path.insert(0, "/opt/trn_rl_repo")
    import concourse.bass as bass
    import concourse.bacc as bacc
    import concourse.mybir as mybir
    import concourse.tile as tile
    from concourse.masks import make_identity

    F32R = mybir.dt.float32r
    F32 = mybir.dt.float32
    I32 = mybir.dt.int32
    M0, M1, M2, M3, M4 = sz["M0"], sz["M1"], sz["M2"], sz["M3"], sz["M4"]
    plans = sz["plans"]  # dict l -> LayerPlan

    nc = bacc.Bacc(
        "TRN2",
        target_bir_lowering=False,
        debug=False,
        enable_asserts=False,
        num_devices=NCORES,
    )

    def din(name, shape, dt=F32R):
        return nc.dram_tensor(name, shape, dt, kind="ExternalInput").ap()

    def dout(name, shape, dt=F32R):
        return nc.dram_tensor(name, shape, dt, kind="ExternalOutput").ap()

    feat = din("feat", [M0, 3])
    featT = din("featT", [3, M0])
    wc = {
        1: din("w0c", [3, 256]),
        2: din("w1c", [256, 256]),
        3: din("w2c", [256, 512]),
        4: din("w3c", [512, 512]),
    }
    def dint(name, shape, dt=F32R):
        return nc.dram_tensor(name, shape, dt, kind="Internal").ap()

    gi_d, vi_d, S_d, wv_d, V_d = {}, {}, {}, {}, {}
    for l, pl in plans.items():
        gi_d[l] = din(f"gi{l}", [pl.chunks * P, 1], I32)
        vi_d[l] = din(f"vi{l}", [pl.ntiles * pl.PF, 1], I32)
        S_d[l] = din(f"S{l}", [pl.ntiles * pl.PF, P])
        wv_d[l] = din(f"wv{l}", [pl.NG * pl.Cin, pl.Cout])
        V_d[l] = dint(f"V{l}", [pl.chunks * P + P, pl.Cout])
    pgx_d = din("pgx", [P, 1], I32)
    pst_d = din("pst", [P, 1], I32)
    x_d = {
        1: dint("x1", [M1, 256]),
        2: dint("x2", [M2, 256]),
        3: dint("x3", [M3, 512]),
        4: dout("x4", [M4, 512]),
    }
    CIN = {1: 3, 2: 256, 3: 256, 4: 512}
    COUT = {1: 256, 2: 256, 3: 512, 4: 512}
    NTILES = {1: M1 // P, 2: M2 // P, 3: M3 // P, 4: M4 // P}
    XIN = {2: x_d[1], 3: x_d[2], 4: x_d[3]}
    VSRC = {1: feat, 2: x_d[1], 3: x_d[2], 4: x_d[3]}

    from contextlib import ExitStack

    with tile.TileContext(nc) as tc, ExitStack() as ctx:
        cpool = ctx.enter_context(tc.tile_pool(name="const", bufs=1))
        pool = ctx.enter_context(tc.tile_pool(name="work", bufs=2))
        spool = ctx.enter_context(tc.tile_pool(name="stage", bufs=2))
        pp = ctx.enter_context(tc.tile_pool(name="psum", bufs=2, space="PSUM"))
        ppm = ctx.enter_context(tc.tile_pool(name="psum_m", bufs=2, space="PSUM"))

        id32 = cpool.tile([P, P], F32, tag="id32")
        make_identity(nc, id32[:])
        ident = cpool.tile([P, P], F32R, tag="ident")
        nc.vector.tensor_copy(out=ident[:], in_=id32[:])

        featT_sb = cpool.tile([3, M0], F32R, tag="featT")
        nc.sync.dma_start(out=featT_sb[:], in_=featT[:, :])
        wcs = {}
        for l in (1, 2, 3, 4):
            cin, cout = CIN[l], COUT[l]
            ncc = max(1, cin // P)
            t = cpool.tile([min(cin, P), ncc * cout], F32R, tag=f"wc{l}")
            for cc in range(ncc):
                nc.sync.dma_start(
                    out=t[:, cc * cout : (cc + 1) * cout],
                    in_=wc[l][cc * P : cc * P + min(cin, P), :],
                )
            wcs[l] = t

        def v_phase(l):
            pl = plans[l]
            cin, cout, ncc = pl.Cin, pl.Cout, max(1, pl.Cin // P)
            src = VSRC[l]
            zt32 = spool.tile([P, cout], F32, tag=f"vz32{l}")
            nc.gpsimd.memset(zt32[:], 0.0)
            zt = spool.tile([P, cout], F32R, tag=f"vz{l}")
            nc.vector.tensor_copy(out=zt[:], in_=zt32[:])
            nc.sync.dma_start(
                out=V_d[l][pl.chunks * P : (pl.chunks + 1) * P, :], in_=zt[:]
            )
            for ch in range(pl.chunks):
                gidx_t = pool.tile([P, 1], I32, tag="gidx")
                nc.sync.dma_start(out=gidx_t[:], in_=gi_d[l][ch * P : (ch + 1) * P, :])
                g = pool.tile([P, cin], F32R, tag=f"vg{l}")
                nc.gpsimd.indirect_dma_start(
                    out=g[:],
                    out_offset=None,
                    in_=src[:, :],
                    in_offset=bass.IndirectOffsetOnAxis(ap=gidx_t[:, :1], axis=0),
                )
                atw = min(cin, P)
                AT = pool.tile([atw, ncc * P], F32R, tag=f"vat{l}")
                for cc in range(ncc):
                    pt = pp.tile([atw, P], F32R, tag="ptr")
                    nc.tensor.transpose(
                        out=pt[:], in_=g[:, cc * atw : cc * atw + atw], identity=ident[:]
                    )
                    nc.vector.tensor_copy(out=AT[:, cc * P : (cc + 1) * P], in_=pt[:])
                wvt = pool.tile([atw, ncc * cout], F32R, tag=f"vwv{l}")
                stage = spool.tile([P, cout], F32R, tag=f"vst{l}")
                for gi_ in range(pl.gpc if pl.SLOT < P else 1):
                    if pl.SLOT < P:
                        gg = ch * pl.gpc + gi_
                        srow = gi_ * pl.SLOT
                        npart = pl.SLOT
                    else:
                        gg = ch // (pl.SLOT // P)
                        srow = 0
                        npart = P
                    for cc in range(ncc):
                        nc.sync.dma_start(
                            out=wvt[:, cc * cout : (cc + 1) * cout],
                            in_=wv_d[l][gg * cin + cc * atw : gg * cin + cc * atw + atw, :],
                        )
                    pv = pp.tile([npart, cout], F32, tag="pv")
                    for cc in range(ncc):
                        nc.tensor.matmul(
                            pv[:],
                            AT[:, cc * P + srow : cc * P + srow + npart],
                            wvt[:, cc * cout : (cc + 1) * cout],
                            start=(cc == 0),
                            stop=(cc == ncc - 1),
                        )
                    nc.vector.tensor_copy(out=stage[srow : srow + npart, :], in_=pv[:])
                nc.sync.dma_start(
                    out=V_d[l][ch * P : (ch + 1) * P, :], in_=stage[:]
                )

        def conv_main(l):
            pl = plans[l]
            cin, cout, ncc = CIN[l], COUT[l], max(1, CIN[l] // P)
            for t in range(NTILES[l]):
                if l == 1:
                    xT = None
                else:
                    xin = pool.tile([P, cin], F32R, tag=f"xin{l}")
                    nc.sync.dma_start(
                        out=xin[:], in_=XIN[l][t * P : (t + 1) * P, :]
                    )
                    xT = pool.tile([P, ncc * P], F32R, tag=f"xT{l}")
                    for cc in range(ncc):
                        pt = pp.tile([P, P], F32R, tag="ptr")
                        nc.tensor.transpose(
                            out=pt[:],
                            in_=xin[:, cc * P : (cc + 1) * P],
                            identity=ident[:],
                        )
                        nc.vector.tensor_copy(
                            out=xT[:, cc * P : (cc + 1) * P], in_=pt[:]
                        )
                spf = min(pl.PF, P)
                St = pool.tile([spf, pl.NS * P], F32R, tag=f"St{l}")
                Vt = pool.tile([spf, pl.NS * cout], F32R, tag=f"Vt{l}")
                for sl in range(pl.NS):
                    r0 = t * pl.PF + sl * spf
                    nc.sync.dma_start(
                        out=St[:, sl * P : (sl + 1) * P], in_=S_d[l][r0 : r0 + spf, :]
                    )
                    vit = pool.tile([P, 1], I32, tag="vit")
                    nc.sync.dma_start(out=vit[:spf], in_=vi_d[l][r0 : r0 + spf, :])
                    nc.gpsimd.indirect_dma_start(
                        out=Vt[:, sl * cout : (sl + 1) * cout],
                        out_offset=None,
                        in_=V_d[l][:, :],
                        in_offset=bass.IndirectOffsetOnAxis(ap=vit[:spf, :1], axis=0),
                    )
                ps = ppm.tile([P, cout], F32, tag="pm")
                for cc in range(ncc):
                    lhsT = (
                        featT_sb[:, t * P : (t + 1) * P]
                        if l == 1
                        else xT[:, cc * P : (cc + 1) * P]
                    )
                    nc.tensor.matmul(
                        ps[:],
                        lhsT,
                        wcs[l][:, cc * cout : (cc + 1) * cout],
                        start=(cc == 0),
                        stop=False,
                    )
                for sl in range(pl.NS):
                    nc.tensor.matmul(
                        ps[:],
                        St[:, sl * P : (sl + 1) * P],
                        Vt[:, sl * cout : (sl + 1) * cout],
                        start=False,
                        stop=(sl == pl.NS - 1),
                    )
                stage = spool.tile([P, cout], F32R, tag=f"st{l}")
                nc.vector.tensor_copy(out=stage[:], in_=ps[:])
                nc.sync.dma_start(out=x_d[l][t * P : (t + 1) * P, :], in_=stage[:])

        def pool_fixup():
            pgx_t = pool.tile([P, 1], I32, tag="pgx")
            pst_t = pool.tile([P, 1], I32, tag="pst")
            nc.sync.dma_start(out=pgx_t[:], in_=pgx_d[:, :])
            nc.sync.dma_start(out=pst_t[:], in_=pst_d[:, :])
            ga = pool.tile([P, 256], F32R, tag="ga")
            gb = pool.tile([P, 256], F32R, tag="gb")
            nc.gpsimd.indirect_dma_start(
                out=ga[:],
                out_offset=None,
                in_=x_d[2][:, :],
                in_offset=bass.IndirectOffsetOnAxis(ap=pgx_t[:, :1], axis=0),
            )
            nc.gpsimd.indirect_dma_start(
                out=gb[:],
                out_offset=None,
                in_=x_d[2][:, :],
                in_offset=bass.IndirectOffsetOnAxis(ap=pst_t[:, :1], axis=0),
            )
            import concourse.mybir as mybir_

            nc.vector.tensor_tensor(
                out=gb[:], in0=gb[:], in1=ga[:], op=mybir_.AluOpType.max
            )
            nc.gpsimd.indirect_dma_start(
                out=x_d[2][:, :],
                out_offset=bass.IndirectOffsetOnAxis(ap=pst_t[:, :1], axis=0),
                in_=gb[:],
                in_offset=None,
            )

        def body():
            v_phase(1)
            conv_main(1)
            v_phase(2)
            conv_main(2)
            pool_fixup()
            v_phase(3)
            conv_main(3)
            v_phase(4)
            conv_main(4)

        if reps == 1:
            body()
        else:
            with tc.For_i(0, reps, 1):
                body()

    nc.compile()
    return nc


# ---------------------------------------------------------------- main entry
def _prepare(coors_np, w_np):
    """All host-side structure building. Returns (sizes, in_maps, meta)."""
    st = build_structures(coors_np)
    Np = st["Np"]
    slab = math.ceil(Np / NCORES)
    cores = [build_core(st, c, slab) for c in range(NCORES)]

    M4 = _round_up(slab, P)
    M3 = _round_up(max(len(cs.L2p) for cs in cores), P)
    M2 = M3 + P
    M1 = M2 + P
    M0 = M1 + P
    sizes = dict(M0=M0, M1=M1, M2=M2, M3=M3, M4=M4)
    for cs in cores:
        assert len(cs.fix_targets) < P
        finish_core(st, cs, sizes)

    plans = {
        1: LayerPlan("c1", 3, 256, M1, [cs.pairs1 for cs in cores]),
        2: LayerPlan("c2", 256, 256, M2, [cs.pairs2 for cs in cores]),
        3: LayerPlan("c3", 256, 512, M3, [cs.pairs3 for cs in cores]),
        4: LayerPlan("c4", 512, 512, M4, [cs.pairs4 for cs in cores]),
    }
    sizes["plans"] = plans

    w_by_l = {1: w_np[0], 2: w_np[1], 3: w_np[2], 4: w_np[3]}
    in_maps = []
    meta = []
    for cs in cores:
        f = np.zeros((M0, 3), np.float32)
        fr = None  # features gathered below
        m = {}
        m["w0c"] = np.ascontiguousarray(w_np[0][KC])
        m["w1c"] = np.ascontiguousarray(w_np[1][KC])
        m["w2c"] = np.ascontiguousarray(w_np[2][KC])
        m["w3c"] = np.ascontiguousarray(w_np[3][KC])
        pairs = {1: cs.pairs1, 2: cs.pairs2, 3: cs.pairs3, 4: cs.pairs4}
        for l, pl in plans.items():
            gidx, vi, S, wv = pl.per_core(pairs[l], w_by_l[l])
            m[f"gi{l}"] = gidx
            m[f"vi{l}"] = vi
            m[f"S{l}"] = S
            m[f"wv{l}"] = wv
        nex = len(cs.fix_extra_rows)
        pgx = np.zeros((P, 1), np.int32)
        pst = np.full((P, 1), M2 - 1, np.int32)
        pgx[:nex, 0] = cs.fix_extra_rows
        pst[:nex, 0] = cs.fix_targets
        m["pgx"], m["pst"] = pgx, pst
        meta.append(cs)
        in_maps.append(m)
    return st, sizes, cores, in_maps


def _fill_features(cores, in_maps, features_np, sizes):
    M0 = sizes["M0"]
    for cs, m in zip(cores, in_maps):
        f = features_np[cs.fvox]  # [M0, 3]
        # zero out rows whose voxel is a dummy duplicate?  not needed: dummy rows
        # produce garbage only in pad outputs.  But halo pads duplicate row0 ->
        # harmless.
        m["feat"] = np.ascontiguousarray(f, np.float32)
        m["featT"] = np.ascontiguousarray(f.T, np.float32)


def kernel(features, w0, w1, w2, w3, coors, batch_size=1):
    if "/opt/trn_rl_repo" not in sys.path:
        sys.path.insert(0, "/opt/trn_rl_repo")
    from concourse import bass_utils

    features = np.asarray(features, np.float32)
    w_np = [np.asarray(w, np.float32) for w in (w0, w1, w2, w3)]
    coors_np = np.asarray(coors, np.int32)
    N = features.shape[0]

    st, sizes, cores, in_maps = _prepare(coors_np, w_np)
    _fill_features(cores, in_maps, features, sizes)
    nc = build_program(sizes)
    res = bass_utils.run_bass_kernel_spmd(nc, in_maps, core_ids=list(range(NCORES)))

    Np = st["Np"]
    x = np.zeros((N, 512), np.float32)
    off = 0
    for cs, r in zip(cores, res.results):
        x[off : off + cs.n_own] = r["x4"][: cs.n_own]
        off += cs.n_own
    assert off == Np
    pcoors = np.zeros((N, 3), np.int32)
    pcoors[:Np] = st["pcoors"]
    return x, pcoors
